# revision 77
# baseline (speedup 1.0000x reference)
"""DeepseekMoE layer on 8 TRN2 NeuronCores — expert-parallel Bass/Tile kernel.

Strategy (self-contained, shapes hardcoded for this problem):
  H=2048, T=2048 tokens, E=16 experts, top-6, I=1408, shared IS=2816.

  Sharding (done on host inside kernel(), per the full-input contract):
    - Router (softmax + top-6) computed on host in fp32 (jax-on-CPU when
      available so near-tie selections match the jax reference bitwise)
      -> per-expert token lists (the "all-to-all dispatch" decision).
    - Core c owns experts 2c, 2c+1: receives w1/w2 transposed for those
      experts plus the gathered+transposed x columns of the tokens routed to
      them (capacity-padded to CAP), and the routing weights.
    - Shared expert is sharded over its intermediate dim: core c owns
      rows [352c, 352c+352) (padded to 384 = 3*128) of the shared MLP.
    - Each core returns per-expert outputs [CAP, H] (pre-scaled by routing
      weights) and a dense shared partial [T, H]; host scatter-adds.

  On-device per expert e (all matmuls fp32r = full PE rate, ~1.5e-4 rms):
    s1:  gate_up.T[o, t] = sum_h w1t[h, o] * xsel[h, t]
         silu fused into PSUM eviction; up-eviction is an in-place multiply
         -> act.T [i, t] in SBUF (fp32r)
    s2:  y[t, h] = sum_i act.T[i, t] * w2t[i, h], eviction fused with
         per-token routing-weight scale (ACT Copy, scale AP).
  Shared expert: identical structure over all T in 1024-token halves with
  its 24KB/partition down-projection weights kept resident.
  Overlap: each block's stage-2 second half is emitted after the next
  block's stage-1 (cross-block software pipeline over split s1/s2 PSUM
  pools), and deep output staging (6 bufs) keeps PSUM eviction off the
  store queue's critical path.
"""

import os
import sys

sys.path.insert(0, "/opt/trn_rl_repo")

import numpy as np

import concourse.bass as bass  # noqa: F401
import concourse.tile as tile
from concourse import bacc, mybir
from concourse.bass_utils import run_bass_kernel_spmd

H = 2048
T = 2048
E = 16
TOPK = 6
I2 = 2816  # 2*I
I = 1408
ISH = 2816  # shared intermediate (per gate/up half)
NCORES = 8
CAP0 = 896  # per-expert token capacity (avg load 768); grown if exceeded
SSL = 352  # shared-intermediate slice per core
SSLP = 384  # padded to 3*128

F32 = mybir.dt.float32
F32R = mybir.dt.float32r
AF = mybir.ActivationFunctionType

_compiled = {}
last_result = None  # BassKernelResults of the most recent run (for profiling)


def _nchunks(n):
    """Split n (multiple of 128) into fp32-matmul-friendly free-dim chunks:
    each <= 512 and >= 256 (fp32r runs 1 cyc/row only at N >= 256)."""
    out = []
    while n > 0:
        if n > 512:
            out.append(512)
            n -= 512
        elif n >= 256 or not out:
            out.append(n)
            n = 0
        else:  # n == 128: rebalance with previous 512 -> 384 + 256
            out[-1] -= 128
            out.append(256)
            n = 0
    return out


def _fine_chunks(ntok):
    """Chunk list with a small (256) first chunk — lets the first PSUM
    group start after a fraction of the x block has landed."""
    return [256] + _nchunks(ntok - 256)


def _emit_s1(nc, pools, *, w1t_ap, x_parts, act_tile, ntok, n_gate_ot,
             first_slab_hipri=False, chunks=None):
    """Stage 1: gate_up.T tiles, silu fused into eviction, in-place up-mul.

    w1t_ap:  DRAM [H, 2*n_gate_ot*128] (gate cols then up cols)
    x_parts: per token-chunk (sbuf_tile, col0) holding that chunk's x.T cols
    act_tile: SBUF [128, n_gate_ot, ntok] fp32r (written here)
    """
    w1p, psp = pools["w1"], pools["ps"]
    KT = 16  # h contraction tiles
    w1t_r = w1t_ap.rearrange("(k p) o -> p k o", p=128)
    spans = []
    t0 = 0
    for tcw in (chunks or _nchunks(ntok)):
        spans.append((t0, tcw))
        t0 += tcw
    assert len(x_parts) == len(spans)
    tc = pools["tc"]
    for ot in range(2 * n_gate_ot):
        w1slab = w1p.tile([128, KT, 128], F32R, tag="w1slab")
        if ot == 0 and first_slab_hipri:
            with tc.high_priority():
                nc.sync.dma_start(out=w1slab[:],
                                  in_=w1t_r[:, :, ot * 128:(ot + 1) * 128])
        else:
            nc.sync.dma_start(out=w1slab[:],
                              in_=w1t_r[:, :, ot * 128:(ot + 1) * 128])
        # k outer / chunk inner: consecutive matmuls reuse the stationary
        # operand w1slab[:, k, :], amortizing its LDWEIGHTS
        pss = [psp.tile([128, 512], F32, tag="ps", name=f"ps1_{ot}_{ci}")
               for ci in range(len(spans))]
        for k in range(KT):
            for ci, (t0, tcw) in enumerate(spans):
                xpt, xc0 = x_parts[ci]
                nc.tensor.matmul(
                    pss[ci][:, :tcw],
                    w1slab[:, k, :],
                    xpt[:, k, xc0:xc0 + tcw],
                    start=(k == 0),
                    stop=(k == KT - 1),
                )
        for ci, (t0, tcw) in enumerate(spans):
            if ot < n_gate_ot:
                nc.scalar.activation(
                    out=act_tile[:, ot, t0:t0 + tcw],
                    in_=pss[ci][:, :tcw],
                    func=AF.Silu,
                )
            else:
                sl = act_tile[:, ot - n_gate_ot, t0:t0 + tcw]
                nc.vector.tensor_mul(sl, pss[ci][:, :tcw], sl)


def _emit_s2(nc, pools, *, act_tile, w2t_ap, out_ap, out_row0, ntok,
             n_gate_ot, cw_tile, cw_col0=0, resident_w2=None, part=0):
    """Stage 2: down proj, per-token scale fused into eviction.

    w2t_ap:  DRAM [n_gate_ot*128, H]
    out_ap:  DRAM output, rows [out_row0, out_row0+ntok), all H cols
    cw_tile: SBUF [128, >=cw_col0+ntok/128] per-token scale, or None
    resident_w2: optional pre-loaded SBUF [128, n_gate_ot, H] weight tile
    """
    w2p, psp, outp = pools["w2"], pools["ps2"], pools["out"]
    s2_k = n_gate_ot
    w2t_r = w2t_ap.rearrange("(k p) h -> p k h", p=128)
    stash = pools.setdefault("w2stash", {})

    def get_slab(hc):
        key = (id(w2t_ap), out_row0, hc)
        if key in stash:
            return stash.pop(key)
        w2slab = w2p.tile([128, s2_k, 512], F32R, tag="w2slab",
                          name=f"w2slab_{out_row0}_{hc}")
        nc.sync.dma_start(out=w2slab[:],
                          in_=w2t_r[:, :, hc * 512:(hc + 1) * 512])
        return w2slab

    def prefetch_slab(hc):
        stash[(id(w2t_ap), out_row0, hc)] = get_slab(hc)
    ntt = ntok // 128
    tt_list = {0: range(ntt), 1: range(ntt // 2), 2: range(ntt // 2, ntt)}[part]
    hc_list = {0: range(4), 1: range(2), 2: range(2, 4)}[part]
    if resident_w2 is not None:
        # tt outer / hc inner: the stationary act[:, k, tt] is reused across
        # all four hc matmuls, amortizing its LDWEIGHTS 4x
        for tt in tt_list:
            pss = [psp.tile([128, 512], F32, tag="ps2", name=f"ps2r_{tt}_{hc}")
                   for hc in range(4)]
            for k in range(s2_k):
                for hc in range(4):
                    nc.tensor.matmul(
                        pss[hc][:],
                        act_tile[:, k, tt * 128:(tt + 1) * 128],
                        resident_w2[:, k, hc * 512:(hc + 1) * 512],
                        start=(k == 0),
                        stop=(k == s2_k - 1),
                    )
            for hc in range(4):
                ysb = outp.tile([128, 512], F32, tag="ysb",
                                name=f"ysbr_{tt}_{hc}")
                # alternate evict engine: ACT and DVE each drain two PSUM
                # groups per tt, halving the slot-recycle critical path
                if hc % 2 == 0:
                    nc.scalar.activation(out=ysb[:], in_=pss[hc][:], func=AF.Copy)
                else:
                    nc.vector.tensor_copy(ysb[:], pss[hc][:])
                nc.sync.dma_start(
                    out=out_ap[out_row0 + tt * 128: out_row0 + (tt + 1) * 128,
                               hc * 512:(hc + 1) * 512],
                    in_=ysb[:],
                )
        return
    for hc in hc_list:
        w2slab = get_slab(hc)
        for tt in range(ntok // 128):
            ps = psp.tile([128, 512], F32, tag="ps2", name=f"ps2_{hc}_{tt}")
            for k in range(s2_k):
                nc.tensor.matmul(
                    ps[:],
                    act_tile[:, k, tt * 128:(tt + 1) * 128],
                    w2slab[:, k, :],
                    start=(k == 0),
                    stop=(k == s2_k - 1),
                )
            ysb = outp.tile([128, 512], F32, tag="ysb", name=f"ysb_{hc}_{tt}")
            if cw_tile is not None:
                nc.scalar.activation(
                    out=ysb[:], in_=ps[:], func=AF.Copy,
                    scale=cw_tile[:, cw_col0 + tt:cw_col0 + tt + 1])
            else:
                nc.scalar.activation(out=ysb[:], in_=ps[:], func=AF.Copy)
            nc.sync.dma_start(
                out=out_ap[out_row0 + tt * 128: out_row0 + (tt + 1) * 128,
                           hc * 512:(hc + 1) * 512],
                in_=ysb[:],
            )



def _build(cap):
    nc = bacc.Bacc("TRN2", target_bir_lowering=False, debug=False)

    aps = {}
    for j in range(2):
        aps[f"xs{j}"] = nc.dram_tensor(f"xs{j}", [H, cap], F32R, kind="ExternalInput").ap()
        aps[f"w1t{j}"] = nc.dram_tensor(f"w1t{j}", [H, I2], F32R, kind="ExternalInput").ap()
        aps[f"w2t{j}"] = nc.dram_tensor(f"w2t{j}", [I, H], F32R, kind="ExternalInput").ap()
        aps[f"cw{j}"] = nc.dram_tensor(f"cw{j}", [cap], F32, kind="ExternalInput").ap()
        aps[f"y{j}"] = nc.dram_tensor(f"y{j}", [cap, H], F32, kind="ExternalOutput").ap()
    aps["xt"] = nc.dram_tensor("xt", [H, T], F32R, kind="ExternalInput").ap()
    aps["sw1t"] = nc.dram_tensor("sw1t", [H, 2 * SSLP], F32R, kind="ExternalInput").ap()
    aps["sw2t"] = nc.dram_tensor("sw2t", [SSLP, H], F32R, kind="ExternalInput").ap()
    aps["ys"] = nc.dram_tensor("ys", [T, H], F32, kind="ExternalOutput").ap()

    # token blocks per expert (<=1024 each, multiples of 128)
    eblocks = []
    r0 = 0
    while r0 < cap:
        w = min(1024, cap - r0)
        eblocks.append((r0, w))
        r0 += w

    import contextlib
    with tile.TileContext(nc) as tc, contextlib.ExitStack() as ctx:
        pools = {
            "x": ctx.enter_context(tc.tile_pool(name="x", bufs=1)),
            # cap > 896 grows the x slot to 64KB/partition; shed one w1
            # prefetch buffer to stay inside SBUF on that fallback path
            "w1": ctx.enter_context(tc.tile_pool(name="w1",
                                                 bufs=4 if cap <= 896 else 2)),
            "w2": ctx.enter_context(tc.tile_pool(name="w2", bufs=2)),
            "act": ctx.enter_context(tc.tile_pool(name="act", bufs=1)),
            "out": ctx.enter_context(tc.tile_pool(name="out", bufs=6)),
            # separate s1/s2 PSUM pools: the cross-block s2 deferral must
            # never be starved of PSUM slots by the next block's stalled s1
            "ps": ctx.enter_context(tc.tile_pool(name="ps", bufs=4, space="PSUM")),
            "ps2": ctx.enter_context(tc.tile_pool(name="ps2", bufs=4, space="PSUM")),
            "misc": ctx.enter_context(tc.tile_pool(name="misc", bufs=2)),
        }

        pools["tc"] = tc
        cw_tiles = {}

        def get_cw(j):  # lazy: cw loads shouldn't precede compute-critical DMAs
            if j not in cw_tiles:
                cw_r = aps[f"cw{j}"].rearrange("(n p) -> p n", p=128)
                cw_tiles[j] = pools["misc"].tile([128, cap // 128], F32,
                                                 tag=f"cw{j}", name=f"cw{j}_t")
                nc.sync.dma_start(out=cw_tiles[j][:], in_=cw_r[:])
            return cw_tiles[j]

        # Block order [e0, sh0, sh1, e1]:
        # - the big xt (shared) transfers land on the clean early boundaries
        #   where the previous block's stage-1 finishes on time
        # - the kernel ends on an expert block, whose store rate stays below
        #   its PE rate, shrinking the end-of-kernel store drain
        def expert_blocks(j):
            xs_r = aps[f"xs{j}"].rearrange("(k p) t -> p k t", p=128)
            return [dict(
                x_src=xs_r[:, :, row0:row0 + ntok], ntok=ntok, n_gate_ot=11,
                w1t_ap=aps[f"w1t{j}"], w2t_ap=aps[f"w2t{j}"],
                out_ap=aps[f"y{j}"], out_row0=row0,
                cw_j=j, cw_col0=row0 // 128, slot="xsel",
            ) for (row0, ntok) in eblocks]

        xt_r = aps["xt"].rearrange("(k p) t -> p k t", p=128)
        shared_blocks = [dict(
            x_src=xt_r[:, :, half * 1024:(half + 1) * 1024], ntok=1024,
            n_gate_ot=3, w1t_ap=aps["sw1t"], w2t_ap=aps["sw2t"],
            out_ap=aps["ys"], out_row0=half * 1024,
            cw_j=None, cw_col0=0, slot="xsel",
        ) for half in range(2)]

        blocks = expert_blocks(0) + expert_blocks(1) + shared_blocks

        def load_x(b, chunks, hipri_first_only=False):
            # chunked at the s1 span boundaries: each s1 PSUM group starts
            # as soon as its own columns have landed. Shared blocks put
            # chunk 0 in a dependency-free aux slot so the next block's
            # stage-1 can start the moment the previous one ends.
            parts = []
            xt_tile = pools["x"].tile([128, 16, b["ntok"]], F32R, tag=b["slot"],
                                      name=f"x_{b['slot']}_{b['out_row0']}")
            t0 = 0
            for ci, tcw in enumerate(chunks):
                if ci == 0 or not hipri_first_only:
                    with tc.high_priority():
                        nc.sync.dma_start(out=xt_tile[:, :, t0:t0 + tcw],
                                          in_=b["x_src"][:, :, t0:t0 + tcw])
                else:
                    nc.sync.dma_start(out=xt_tile[:, :, t0:t0 + tcw],
                                      in_=b["x_src"][:, :, t0:t0 + tcw])
                parts.append((xt_tile, t0))
                t0 += tcw
            return parts

        # Emit s1(n), then block n+1's x-load, then s2(n): the next x-load
        # lands ahead of s2(n)'s weight slabs in the scheduler's priority
        # order, so its (large) transfer overlaps s2(n) compute instead of
        # queueing behind it in the DGE FIFO.
        preloaded_aux = {}
        shared_w2_res = [None]

        def load_aux(b):
            aux = pools["x"].tile([128, 16, 512], F32R, tag="xaux",
                                  name=f"xaux_{b['out_row0']}")
            nc.sync.dma_start(out=aux[:], in_=b["x_src"][:, :, 0:512])
            return aux

        def s1_chunks(n):
            return (_fine_chunks(blocks[n]["ntok"]) if n == 0
                    else _nchunks(blocks[n]["ntok"]))

        def emit_s2_part(b, act_tile, part):
            res_w2 = None
            if b["cw_j"] is None:  # shared expert: 24KB w2 slice kept resident
                if shared_w2_res[0] is None:
                    rt = pools["w2"].tile([128, 3, H], F32R, tag="w2slab",
                                          name="sw2_resident")
                    nc.sync.dma_start(
                        out=rt[:],
                        in_=b["w2t_ap"].rearrange("(k p) h -> p k h", p=128))
                    shared_w2_res[0] = rt
                res_w2 = shared_w2_res[0]
            _emit_s2(nc, pools, act_tile=act_tile, w2t_ap=b["w2t_ap"],
                     out_ap=b["out_ap"], out_row0=b["out_row0"],
                     ntok=b["ntok"], n_gate_ot=b["n_gate_ot"],
                     cw_tile=None if b["cw_j"] is None else get_cw(b["cw_j"]),
                     cw_col0=b["cw_col0"], resident_w2=res_w2, part=part)

        x_tiles = [load_x(blocks[0], s1_chunks(0), hipri_first_only=True)]
        deferred = None
        for n, b in enumerate(blocks):
            act_tile = pools["act"].tile([128, b["n_gate_ot"], b["ntok"]],
                                         F32R, tag="act")
            _emit_s1(nc, pools, w1t_ap=b["w1t_ap"], x_parts=x_tiles[n],
                     act_tile=act_tile, ntok=b["ntok"],
                     n_gate_ot=b["n_gate_ot"], first_slab_hipri=True,
                     chunks=s1_chunks(n))
            if n + 1 < len(blocks):
                x_tiles.append(load_x(blocks[n + 1], s1_chunks(n + 1)))
            # cross-block software pipeline: the previous block's deferred
            # s2 half sits after this block's s1 in priority order, so the
            # scheduler can fill this block's x/slab wait with it
            if deferred is not None:
                emit_s2_part(*deferred, part=2)
                deferred = None
            emit_s2_part(b, act_tile, part=1)
            deferred = (b, act_tile)
        if deferred is not None:
            emit_s2_part(*deferred, part=2)

    nc.compile()
    return nc


def _route(xf, gate_w):
    """Host router: fp32 softmax + top-6.

    Uses jax on CPU when available so selection/weights match the jax
    reference bit-for-bit (matters only for near-exact prob ties).
    """
    try:
        import jax
        import jax.numpy as jnp

        cpu = jax.devices("cpu")[0]
        with jax.default_device(cpu):
            logits = jnp.asarray(xf) @ jnp.asarray(gate_w).T
            probs = jax.nn.softmax(logits.astype(jnp.float32), axis=-1)
            _, sel = jax.lax.top_k(probs, TOPK)
        return np.asarray(probs), np.asarray(sel)
    except Exception:
        logits = xf @ gate_w.T  # [T, E] fp32
        m = logits.max(axis=-1, keepdims=True)
        e = np.exp(logits - m, dtype=np.float32)
        probs = e / e.sum(axis=-1, keepdims=True)
        sel = np.argsort(-probs, axis=-1, kind="stable")[:, :TOPK]
        return probs, sel


def kernel(x, gate_w, w1, w2, shared_w1, shared_w2):
    x = np.asarray(x, np.float32)
    gate_w = np.asarray(gate_w, np.float32)
    w1 = np.asarray(w1, np.float32)
    w2 = np.asarray(w2, np.float32)
    shared_w1 = np.asarray(shared_w1, np.float32)
    shared_w2 = np.asarray(shared_w2, np.float32)

    B, S, Hd = x.shape
    xf = np.ascontiguousarray(x.reshape(-1, Hd))  # [T, H]

    probs, sel = _route(xf, gate_w)
    onehot = np.zeros((T, E), bool)
    onehot[np.arange(T)[:, None], sel] = True
    idx_e = [np.nonzero(onehot[:, e])[0] for e in range(E)]
    counts = np.array([len(ix) for ix in idx_e])

    cap = CAP0
    while counts.max() > cap:
        cap += 128
    if cap not in _compiled:
        _compiled[cap] = _build(cap)
    nc = _compiled[cap]

    xt = np.ascontiguousarray(xf.T)  # [H, T]

    in_maps = []
    for c in range(NCORES):
        m = {"xt": xt}
        for j in range(2):
            e = 2 * c + j
            ix = idx_e[e]
            xs = np.zeros((cap, H), np.float32)
            xs[: len(ix)] = xf[ix]
            m[f"xs{j}"] = np.ascontiguousarray(xs.T)
            m[f"w1t{j}"] = np.ascontiguousarray(w1[e].T)
            m[f"w2t{j}"] = np.ascontiguousarray(w2[e].T)
            cw = np.zeros(cap, np.float32)
            cw[: len(ix)] = probs[ix, e]
            m[f"cw{j}"] = cw
        sw1t = np.zeros((H, 2 * SSLP), np.float32)
        sw1t[:, :SSL] = shared_w1[SSL * c: SSL * (c + 1)].T
        sw1t[:, SSLP: SSLP + SSL] = shared_w1[ISH + SSL * c: ISH + SSL * (c + 1)].T
        m["sw1t"] = sw1t
        sw2t = np.zeros((SSLP, H), np.float32)
        sw2t[:SSL] = shared_w2[:, SSL * c: SSL * (c + 1)].T
        m["sw2t"] = sw2t
        in_maps.append(m)

    try:
        res = run_bass_kernel_spmd(nc, in_maps, list(range(NCORES)))
    except ModuleNotFoundError:
        # BASS_TRACE=1 requires the axon NTFF hook (antenv.axon_hooks),
        # absent in some containers — retry with tracing disabled.
        os.environ["BASS_NEVER_TRACE"] = "1"
        res = run_bass_kernel_spmd(nc, in_maps, list(range(NCORES)))
    global last_result
    last_result = res

    out = np.zeros((T, H), np.float32)
    for c in range(NCORES):
        out += res.results[c]["ys"]
        for j in range(2):
            e = 2 * c + j
            ix = idx_e[e]
            out[ix] += res.results[c][f"y{j}"][: len(ix)]

    return out.reshape(B, S, Hd)
